# revision 3
# baseline (speedup 1.0000x reference)
"""Trainium2 Bass kernel v3 for the Dale CB-cell step — loop-structured.

Per batch column b (H=48, IN=8):
    v      = hidden[b, :]
    r      = sigmoid(v)
    zpre   = Ksp @ r + P_z @ x[:, b] + b_z
    u      = DT*(W @ r + P_masked @ x[:, b] + b_v)
    v_new  = v * (1 - DT*sigmoid(zpre)) + u

Key measured fact on this stack: unrolled instructions cost ~40-50 us
each (global dispatch tax), but instructions inside a tc.For_i hardware
loop cost only ~5 us. So v3 wraps the whole per-core body in one For_i
over 16384-row macros (8 iterations), with everything inside the loop
body (81 engine instructions, fits IRAM).

x enters via a host-side logit trick so it can ride the same
load+sigmoid path as hidden: hidx[:, 48:56] = logit((x-lo)/(hi-lo)),
hidx[:, 56] = 30 (sigmoid -> 1.0 = bias row); the (hi-lo)/lo affine is
folded into the weight block. This removes the separate x-inject DMA.

Per macro body: 1 DMA load (dynamic offset), 1 ACT sigmoid (f32->bf16),
1 xbar transpose to H-major, 8x(4 matmuls (64,96)x(64,512) + 1 DVE
psum evict), 1 xbar transpose back, 4-op f32 epilogue, 1 DMA store.
"""

import sys

if "/opt/trn_rl_repo" not in sys.path:
    sys.path.insert(0, "/opt/trn_rl_repo")

import numpy as np

H = 48
IN = 8
DT = 0.1
B = 1048576
N_CORES = 8
B_CORE = B // N_CORES          # 131072
M_R = 16384                    # rows per macro (outer loop step)
N_CHUNK = M_R // 128           # 128 transpose chunks per macro
GRP = 2048                     # batch cols per psum group (4 banks)
N_GRP = M_R // GRP             # 8 matmul groups per macro

_NC_CACHE = {}


def _softplus64(x):
    x = x.astype(np.float64)
    return np.log1p(np.exp(-np.abs(x))) + np.maximum(x, 0.0)


def _build_wblk(P, b_v, K, C, P_z, b_z, e_e, e_i, lo, hi):
    """Fold all weights into the (64, 96) matmul lhsT block (float64)."""
    Ksp = _softplus64(K)
    Csp = _softplus64(C)
    S = Ksp + Csp
    e_e = float(np.asarray(e_e).reshape(-1)[0])
    e_i = float(np.asarray(e_i).reshape(-1)[0])
    W_E = np.maximum(e_e * S[:, : H // 2], 0.0)
    W_I = -np.maximum(-(e_i * S[:, H // 2 :]), 0.0)
    W = np.concatenate([W_E, W_I], axis=1)          # (H, H)
    rows = np.arange(H)
    keep = ~(((rows >= H // 4) & (rows < H // 2)) | (rows >= 3 * H // 4))
    P_masked = P.astype(np.float64) * keep[:, None]
    P_z = P_z.astype(np.float64)
    span = hi - lo

    blk = np.zeros((64, 96), np.float64)
    blk[0:H, 0:H] = Ksp.T                         # z half: Ksp @ r
    blk[0:H, H : 2 * H] = (DT * W).T              # u half: DT * W @ r
    blk[H : H + IN, 0:H] = (span * P_z).T         # z: P_z @ x via x-tilde
    blk[H : H + IN, H : 2 * H] = (DT * span * P_masked).T
    blk[H + IN, 0:H] = b_z.astype(np.float64).reshape(-1) + lo * P_z.sum(axis=1)
    blk[H + IN, H : 2 * H] = DT * (
        b_v.astype(np.float64).reshape(-1) + lo * P_masked.sum(axis=1)
    )
    return blk


def _strip_redundant_ldweights(nc):
    """Remove loop-body InstLdweights that reload the same constant weights.

    The PE array state persists across matmuls; only the first LDW (which
    carries the w_sb DMA wait) is kept per block. All matmuls use the same
    (64, 96) stationary block, so later reloads are pure dispatch overhead.
    """
    import concourse.mybir as mybir

    removed = 0
    for blk in nc.m.functions[0].blocks:
        insts = blk.instructions
        seen = False
        keep = []
        blk_removed = 0
        for inst in insts:
            if isinstance(inst, mybir.InstLdweights):
                if seen and not inst.has_wait() and not inst.has_update():
                    blk_removed += 1
                    continue
                seen = True
            keep.append(inst)
        if blk_removed:
            del insts[:]
            insts.extend(keep)
            removed += blk_removed
    return removed


def _build_nc_v3(b_core, reps=1, bench=False, strip_ldw=True, grp=GRP, ps_bufs=1, staggered=False, unroll=1, act_evict=True, h16=False):
    import concourse.bacc as bacc
    import concourse.mybir as mybir
    import concourse.tile as tile
    from concourse.bass import ds

    F32 = mybir.dt.float32
    BF16 = mybir.dt.bfloat16
    SIG = mybir.ActivationFunctionType.Sigmoid

    HDT = BF16 if h16 else F32
    nc = bacc.Bacc("TRN2", target_bir_lowering=False, debug=False)
    big = "Internal" if bench else None
    hidx = nc.dram_tensor("hidx", [b_core, 64], HDT, kind=big or "ExternalInput")
    wdram = nc.dram_tensor("wdram", [64, 96], BF16, kind="ExternalInput")
    out = nc.dram_tensor("out", [b_core, H], F32, kind=big or "ExternalOutput")
    dbg = nc.dram_tensor("dbg", [128, 64], F32, kind="ExternalOutput") if bench else None

    with tile.TileContext(nc) as tc:
        with (
            tc.tile_pool(name="sb", bufs=1) as sb,
            tc.tile_pool(name="psum", bufs=1, space="PSUM") as pp,
        ):
            w_sb = sb.tile([64, 96], BF16)
            nc.sync.dma_start(w_sb[:], wdram[:])

            hvx = sb.tile([128, N_CHUNK * 64], HDT, name="hvx", tag="hvx")
            rbm = sb.tile([128, M_R], BF16, name="rbm", tag="rbm")
            trans = sb.tile([128, M_R], BF16, name="trans", tag="trans")
            zu = sb.tile([128, M_R], BF16, name="zu", tag="zu")
            zuT = sb.tile([128, M_R], BF16, name="zuT", tag="zuT")
            acc = sb.tile([128, N_CHUNK * H], F32, name="acc", tag="acc")
            n_grp = M_R // grp
            ps_tiles = [
                pp.tile([96, grp], F32, name=f"ps{i}", tag=f"ps{i}")
                for i in range(ps_bufs)
            ]

            hvx3 = hvx[:].rearrange("p (c e) -> p c e", e=64)
            rb3 = rbm[:].rearrange("p (c e) -> p c e", e=128)[:, :, 0:64]
            tr3 = trans[:].rearrange("p (c e) -> p c e", e=128)
            zt3 = zuT[:].rearrange("p (c e) -> p c e", e=128)
            zview = zuT[:].rearrange("p (c e) -> p c e", e=128)[:, :, 0:H]
            uview = zuT[:].rearrange("p (c e) -> p c e", e=128)[:, :, H : 2 * H]
            acc3 = acc[:].rearrange("p (c h) -> p c h", h=H)

            def emit_macro(row0):
                nc.sync.dma_start(
                    hvx3,
                    hidx[ds(row0, M_R), :].rearrange("(p c) e -> p c e", c=N_CHUNK),
                )
                nc.scalar.activation(rb3, hvx3, SIG)
                nc.sync.dma_start(tr3, rbm[:], transpose=True)

                for g in range(n_grp):
                    ps = ps_tiles[g % ps_bufs]
                    for s in range(grp // 512):
                        c0 = grp * g + 512 * s
                        nc.tensor.matmul(
                            ps[:, 512 * s : 512 * s + 512],
                            w_sb[:],
                            trans[0:64, c0 : c0 + 512],
                            start=True,
                            stop=True,
                        )
                    if act_evict:
                        nc.scalar.activation(
                            zu[0:96, grp * g : grp * g + grp], ps[:],
                            mybir.ActivationFunctionType.Copy,
                        )
                    else:
                        nc.vector.tensor_copy(
                            zu[0:96, grp * g : grp * g + grp], ps[:]
                        )

                nc.sync.dma_start(zt3, zu[:], transpose=True)
                nc.scalar.activation(acc3, zview, SIG)
                nc.vector.tensor_scalar(
                    acc[:], acc[:], -DT, 1.0,
                    mybir.AluOpType.mult, mybir.AluOpType.add,
                )
                nc.vector.tensor_mul(acc3, hvx3[:, :, 0:H], acc3)
                nc.vector.tensor_tensor(
                    acc3, acc3, uview, op=mybir.AluOpType.add
                )
                nc.sync.dma_start(
                    out[ds(row0, M_R), :].rearrange("(p c) h -> p c h", c=N_CHUNK),
                    acc3,
                )

            for _ in range(reps):
                with tc.For_i(0, b_core, unroll * M_R,
                              staggered_reset=staggered) as mo:
                    for u in range(unroll):
                        emit_macro(mo + u * M_R if u else mo)

            if bench:
                dbg_t = sb.tile([128, 64], F32, name="dbg_t", tag="dbg_t")
                nc.gpsimd.memset(dbg_t[:], 0.0)
                nc.sync.dma_start(dbg[:], dbg_t[:])

    if strip_ldw:
        _strip_redundant_ldweights(nc)
    nc.compile()
    return nc


def get_nc_v3(b_core=B_CORE, reps=1, bench=False):
    key = ("v3", b_core, reps, bench)
    if key not in _NC_CACHE:
        _NC_CACHE[key] = _build_nc_v3(b_core, reps, bench)
    return _NC_CACHE[key]


def prepare_inputs_v3(hidden, x, P, b_v, K, C, P_z, b_z, e_e, e_i,
                      n_cores=N_CORES, h16=False):
    import ml_dtypes

    bf16 = ml_dtypes.bfloat16
    hidden = np.asarray(hidden, np.float32)
    x = np.asarray(x, np.float64)                  # (IN, B)
    lo = float(x.min()) - 1e-3
    hi = float(x.max()) + 1e-3
    xt = (x - lo) / (hi - lo)
    t = np.log(xt / (1.0 - xt)).astype(np.float32)  # logit, (IN, B)

    blk = _build_wblk(
        np.asarray(P), np.asarray(b_v), np.asarray(K), np.asarray(C),
        np.asarray(P_z), np.asarray(b_z), np.asarray(e_e), np.asarray(e_i),
        lo, hi,
    ).astype(bf16)

    btot = hidden.shape[0]
    hidx = np.zeros((btot, 64), bf16 if h16 else np.float32)
    hidx[:, 0:H] = hidden.astype(bf16) if h16 else hidden
    hidx[:, H : H + IN] = t.T.astype(bf16) if h16 else t.T
    hidx[:, H + IN] = 30.0                         # sigmoid -> 1.0 bias row

    b_core = btot // n_cores
    in_maps = []
    for k in range(n_cores):
        s = slice(k * b_core, (k + 1) * b_core)
        in_maps.append({"hidx": np.ascontiguousarray(hidx[s]), "wdram": blk})
    return in_maps


def kernel(hidden, x, P, b_v, K, C, P_z, b_z, e_e, e_i):
    from concourse.bass_utils import run_bass_kernel_spmd

    nc = get_nc_v3(B_CORE)
    in_maps = prepare_inputs_v3(hidden, x, P, b_v, K, C, P_z, b_z, e_e, e_i)
    res = run_bass_kernel_spmd(nc, in_maps, list(range(N_CORES)))
    out = np.concatenate([r["out"] for r in res.results], axis=0)
    return out.astype(np.float32)


# revision 4
# speedup vs baseline: 1.5941x; 1.5941x over previous
"""Trainium2 Bass kernel v3 for the Dale CB-cell step — loop-structured.

Per batch column b (H=48, IN=8):
    v      = hidden[b, :]
    r      = sigmoid(v)
    zpre   = Ksp @ r + P_z @ x[:, b] + b_z
    u      = DT*(W @ r + P_masked @ x[:, b] + b_v)
    v_new  = v * (1 - DT*sigmoid(zpre)) + u

Key measured fact on this stack: unrolled instructions cost ~40-50 us
each (global dispatch tax), but instructions inside a tc.For_i hardware
loop cost only ~5 us. So v3 wraps the whole per-core body in one For_i
over 16384-row macros (8 iterations), with everything inside the loop
body (81 engine instructions, fits IRAM).

x enters via a host-side logit trick so it can ride the same
load+sigmoid path as hidden: hidx[:, 48:56] = logit((x-lo)/(hi-lo)),
hidx[:, 56] = 30 (sigmoid -> 1.0 = bias row); the (hi-lo)/lo affine is
folded into the weight block. This removes the separate x-inject DMA.

Per macro body: 1 DMA load (dynamic offset), 1 ACT sigmoid (f32->bf16),
1 xbar transpose to H-major, 8x(4 matmuls (64,96)x(64,512) + 1 DVE
psum evict), 1 xbar transpose back, 4-op f32 epilogue, 1 DMA store.
"""

import sys

if "/opt/trn_rl_repo" not in sys.path:
    sys.path.insert(0, "/opt/trn_rl_repo")

import numpy as np

H = 48
IN = 8
DT = 0.1
B = 1048576
N_CORES = 8
B_CORE = B // N_CORES          # 131072
M_R = 16384                    # rows per macro (outer loop step)
N_CHUNK = M_R // 128           # 128 transpose chunks per macro
GRP = 2048                     # batch cols per psum group (4 banks)
N_GRP = M_R // GRP             # 8 matmul groups per macro

_NC_CACHE = {}


def _softplus64(x):
    x = x.astype(np.float64)
    return np.log1p(np.exp(-np.abs(x))) + np.maximum(x, 0.0)


def _build_wblk(P, b_v, K, C, P_z, b_z, e_e, e_i, lo, hi):
    """Fold all weights into the (64, 96) matmul lhsT block (float64)."""
    Ksp = _softplus64(K)
    Csp = _softplus64(C)
    S = Ksp + Csp
    e_e = float(np.asarray(e_e).reshape(-1)[0])
    e_i = float(np.asarray(e_i).reshape(-1)[0])
    W_E = np.maximum(e_e * S[:, : H // 2], 0.0)
    W_I = -np.maximum(-(e_i * S[:, H // 2 :]), 0.0)
    W = np.concatenate([W_E, W_I], axis=1)          # (H, H)
    rows = np.arange(H)
    keep = ~(((rows >= H // 4) & (rows < H // 2)) | (rows >= 3 * H // 4))
    P_masked = P.astype(np.float64) * keep[:, None]
    P_z = P_z.astype(np.float64)
    span = hi - lo

    blk = np.zeros((64, 96), np.float64)
    blk[0:H, 0:H] = Ksp.T                         # z half: Ksp @ r
    blk[0:H, H : 2 * H] = (DT * W).T              # u half: DT * W @ r
    blk[H : H + IN, 0:H] = (span * P_z).T         # z: P_z @ x via x-tilde
    blk[H : H + IN, H : 2 * H] = (DT * span * P_masked).T
    blk[H + IN, 0:H] = b_z.astype(np.float64).reshape(-1) + lo * P_z.sum(axis=1)
    blk[H + IN, H : 2 * H] = DT * (
        b_v.astype(np.float64).reshape(-1) + lo * P_masked.sum(axis=1)
    )
    return blk


def _strip_redundant_ldweights(nc):
    """Remove loop-body InstLdweights that reload the same constant weights.

    The PE array state persists across matmuls; only the first LDW (which
    carries the w_sb DMA wait) is kept per block. All matmuls use the same
    (64, 96) stationary block, so later reloads are pure dispatch overhead.
    """
    import concourse.mybir as mybir

    removed = 0
    for blk in nc.m.functions[0].blocks:
        insts = blk.instructions
        seen = False
        keep = []
        blk_removed = 0
        for inst in insts:
            if isinstance(inst, mybir.InstLdweights):
                if seen and not inst.has_wait() and not inst.has_update():
                    blk_removed += 1
                    continue
                seen = True
            keep.append(inst)
        if blk_removed:
            del insts[:]
            insts.extend(keep)
            removed += blk_removed
    return removed


def _build_nc_v3(b_core, reps=1, bench=False, strip_ldw=True, grp=GRP, ps_bufs=1, staggered=False, unroll=1, act_evict=True, h16=False, evict_split=False, zt96=True):
    import concourse.bacc as bacc
    import concourse.mybir as mybir
    import concourse.tile as tile
    from concourse.bass import ds

    F32 = mybir.dt.float32
    BF16 = mybir.dt.bfloat16
    SIG = mybir.ActivationFunctionType.Sigmoid

    HDT = BF16 if h16 else F32
    nc = bacc.Bacc("TRN2", target_bir_lowering=False, debug=False)
    big = "Internal" if bench else None
    hidx = nc.dram_tensor("hidx", [b_core, 64], HDT, kind=big or "ExternalInput")
    wdram = nc.dram_tensor("wdram", [64, 96], BF16, kind="ExternalInput")
    out = nc.dram_tensor("out", [b_core, H], F32, kind=big or "ExternalOutput")
    dbg = nc.dram_tensor("dbg", [128, 64], F32, kind="ExternalOutput") if bench else None

    with tile.TileContext(nc) as tc:
        with (
            tc.tile_pool(name="sb", bufs=1) as sb,
            tc.tile_pool(name="psum", bufs=1, space="PSUM") as pp,
        ):
            w_sb = sb.tile([64, 96], BF16)
            nc.sync.dma_start(w_sb[:], wdram[:])

            hvx = sb.tile([128, N_CHUNK * 64], HDT, name="hvx", tag="hvx")
            rbm = sb.tile([128, M_R], BF16, name="rbm", tag="rbm")
            trans = sb.tile([128, M_R], BF16, name="trans", tag="trans")
            zu = sb.tile([128, M_R], BF16, name="zu", tag="zu")
            EW = 96 if zt96 else 128
            zuT = sb.tile([128, N_CHUNK * EW], BF16, name="zuT", tag="zuT")
            acc = sb.tile([128, N_CHUNK * H], F32, name="acc", tag="acc")
            n_grp = M_R // grp
            ps_tiles = [
                pp.tile([96, grp], F32, name=f"ps{i}", tag=f"ps{i}")
                for i in range(ps_bufs)
            ]

            hvx3 = hvx[:].rearrange("p (c e) -> p c e", e=64)
            rb3 = rbm[:].rearrange("p (c e) -> p c e", e=128)[:, :, 0:64]
            tr3 = trans[:].rearrange("p (c e) -> p c e", e=128)
            zt3 = zuT[:].rearrange("p (c e) -> p c e", e=EW)
            zview = zuT[:].rearrange("p (c e) -> p c e", e=EW)[:, :, 0:H]
            uview = zuT[:].rearrange("p (c e) -> p c e", e=EW)[:, :, H : 2 * H]
            acc3 = acc[:].rearrange("p (c h) -> p c h", h=H)

            def emit_macro(row0):
                nc.sync.dma_start(
                    hvx3,
                    hidx[ds(row0, M_R), :].rearrange("(p c) e -> p c e", c=N_CHUNK),
                )
                nc.scalar.activation(rb3, hvx3, SIG)
                nc.sync.dma_start(tr3, rbm[:], transpose=True)

                for g in range(n_grp):
                    ps = ps_tiles[g % ps_bufs]
                    for s in range(grp // 512):
                        c0 = grp * g + 512 * s
                        nc.tensor.matmul(
                            ps[:, 512 * s : 512 * s + 512],
                            w_sb[:],
                            trans[0:64, c0 : c0 + 512],
                            start=True,
                            stop=True,
                        )
                    use_act = act_evict and (not evict_split or g % 2 == 0)
                    if use_act:
                        nc.scalar.activation(
                            zu[0:96, grp * g : grp * g + grp], ps[:],
                            mybir.ActivationFunctionType.Copy,
                        )
                    else:
                        nc.vector.tensor_copy(
                            zu[0:96, grp * g : grp * g + grp], ps[:]
                        )

                nc.sync.dma_start(zt3, zu[0:96, :] if zt96 else zu[:],
                                  transpose=True)
                nc.scalar.activation(acc3, zview, SIG)
                nc.vector.tensor_scalar(
                    acc[:], acc[:], -DT, 1.0,
                    mybir.AluOpType.mult, mybir.AluOpType.add,
                )
                nc.vector.tensor_mul(acc3, hvx3[:, :, 0:H], acc3)
                nc.vector.tensor_tensor(
                    acc3, acc3, uview, op=mybir.AluOpType.add
                )
                nc.sync.dma_start(
                    out[ds(row0, M_R), :].rearrange("(p c) h -> p c h", c=N_CHUNK),
                    acc3,
                )

            for _ in range(reps):
                with tc.For_i(0, b_core, unroll * M_R,
                              staggered_reset=staggered) as mo:
                    for u in range(unroll):
                        emit_macro(mo + u * M_R if u else mo)

            if bench:
                dbg_t = sb.tile([128, 64], F32, name="dbg_t", tag="dbg_t")
                nc.gpsimd.memset(dbg_t[:], 0.0)
                nc.sync.dma_start(dbg[:], dbg_t[:])

    if strip_ldw:
        _strip_redundant_ldweights(nc)
    nc.compile()
    return nc


def get_nc_v3(b_core=B_CORE, reps=1, bench=False):
    key = ("v3", b_core, reps, bench)
    if key not in _NC_CACHE:
        _NC_CACHE[key] = _build_nc_v3(b_core, reps, bench)
    return _NC_CACHE[key]


def prepare_inputs_v3(hidden, x, P, b_v, K, C, P_z, b_z, e_e, e_i,
                      n_cores=N_CORES, h16=False):
    import ml_dtypes

    bf16 = ml_dtypes.bfloat16
    hidden = np.asarray(hidden, np.float32)
    x = np.asarray(x, np.float64)                  # (IN, B)
    lo = float(x.min()) - 1e-3
    hi = float(x.max()) + 1e-3
    xt = (x - lo) / (hi - lo)
    t = np.log(xt / (1.0 - xt)).astype(np.float32)  # logit, (IN, B)

    blk = _build_wblk(
        np.asarray(P), np.asarray(b_v), np.asarray(K), np.asarray(C),
        np.asarray(P_z), np.asarray(b_z), np.asarray(e_e), np.asarray(e_i),
        lo, hi,
    ).astype(bf16)

    btot = hidden.shape[0]
    hidx = np.zeros((btot, 64), bf16 if h16 else np.float32)
    hidx[:, 0:H] = hidden.astype(bf16) if h16 else hidden
    hidx[:, H : H + IN] = t.T.astype(bf16) if h16 else t.T
    hidx[:, H + IN] = 30.0                         # sigmoid -> 1.0 bias row

    b_core = btot // n_cores
    in_maps = []
    for k in range(n_cores):
        s = slice(k * b_core, (k + 1) * b_core)
        in_maps.append({"hidx": np.ascontiguousarray(hidx[s]), "wdram": blk})
    return in_maps


def kernel(hidden, x, P, b_v, K, C, P_z, b_z, e_e, e_i):
    from concourse.bass_utils import run_bass_kernel_spmd

    nc = get_nc_v3(B_CORE)
    in_maps = prepare_inputs_v3(hidden, x, P, b_v, K, C, P_z, b_z, e_e, e_i)
    res = run_bass_kernel_spmd(nc, in_maps, list(range(N_CORES)))
    out = np.concatenate([r["out"] for r in res.results], axis=0)
    return out.astype(np.float32)
